# revision 20
# baseline (speedup 1.0000x reference)
"""Grouped Conv1d (B=4, T=512, G=129, F=96 -> O=96, K=3, pad=1) on 8 trn2 cores.

Sharding: 129 groups = 16 full groups per core + group 128 split across all
8 cores by (batch b = core//2, T-half = core%2).  SPMD: every core runs the
identical program on its own slice.

Per (group, batch): out[o, t] = sum_k w_k[f, o].T @ x[f, t+k-1]  (3 matmuls
accumulated in fp32 PSUM).  x and w are cast to fp16 on the host: fp16 runs
the PE moving operand at full rate, halves the x DMA bytes, and keeps max
rel err ~5e-4 (accumulate stays fp32).  Bias is added fp32 on ScalarE /
VectorE (alternating) while copying PSUM -> SBUF; output stored fp16.

Layout: one flat x tile [96, 33154] covering all 16 padded (g,b) units plus
the tail-group slice lives in SBUF for the whole kernel (66 KB/partition).
DMA rides the two HWDGE rings with long per-partition lines (first pieces
small for latency, then 16 KB lines = 4x 4096B packets + 64B runt, ~2%
packet overhead instead of the 20% a 4112B line pays).  Output stores are
exactly 4096B/partition per group.  A short burst of dummy matmuls on a
memset scratch tile runs during the first-DMA latency window so the PE HAM
clock-gate is warm (2.4 GHz) when real matmuls start.
"""

from contextlib import ExitStack

import numpy as np

import concourse.bass as bass
import concourse.mybir as mybir
import concourse.tile as tile
from concourse import bacc
from concourse.bass_utils import run_bass_kernel_spmd

B, T, G, F, O, K = 4, 512, 129, 96, 96, 3
NCORES = 8
GPC = 16          # full groups per core (8*16 = 128; group 128 is split 8 ways)
TP = T + 2        # padded unit length per (g, b)
TE = T // 2       # tail-group T chunk per core
TEP = TE + 2
XC = GPC * B * TP + TEP      # x columns per core (33154)
OC = GPC * B * T + TE        # out columns per core (33024)
WC = (GPC + 1) * K * O       # weight columns per core (4896)
NDUMMY = 6                   # HAM warm-up matmuls


def build_program():
    nc = bacc.Bacc("TRN2", target_bir_lowering=False, debug=False,
                   num_devices=NCORES)

    f32 = mybir.dt.float32
    f16 = mybir.dt.float16

    xm = nc.dram_tensor("xm", [F, XC], f16, kind="ExternalInput")
    wt = nc.dram_tensor("wt", [F, WC], f16, kind="ExternalInput")
    bt = nc.dram_tensor("bt", [O, GPC + 1], f32, kind="ExternalInput")
    om = nc.dram_tensor("om", [O, OC], f16, kind="ExternalOutput")

    with ExitStack() as ctx:
        tc = ctx.enter_context(tile.TileContext(nc))
        wpool = ctx.enter_context(tc.tile_pool(name="w", bufs=1))
        opool = ctx.enter_context(tc.tile_pool(name="o", bufs=3))
        pspool = ctx.enter_context(tc.tile_pool(name="ps", bufs=8, space="PSUM"))

        x_sb = wpool.tile([F, XC], f16)
        w_sb = wpool.tile([F, WC], f16)
        b_sb = wpool.tile([O, GPC + 1], f32)
        scr = wpool.tile([F, O + T], f16)      # dummy-matmul operands

        nc.vector.memset(scr[:], 0.0)

        def ldu(eng, u0, u1):                  # load x units [u0, u1)
            eng.dma_start(x_sb[:, u0 * TP:u1 * TP], xm[:, u0 * TP:u1 * TP])

        kw = K * O
        # prologue: at most 8 outstanding HWDGE DMAs (8 completion-sem
        # lanes exist; a 9th dma_start would stall its whole engine queue
        # on lane reuse).  Tiny interleaved first pieces so the early
        # per-ring completion pipeline (one sem per ~1us) matches the
        # cold-matmul consumption order; later stripes are issued from
        # inside the compute loop on the sync ring only — sync runs no
        # compute, so stripe issues never queue behind a bias-add wait
        nc.sync.dma_start(w_sb[:, :kw], wt[:, :kw])          # g0 weights
        ldu(nc.scalar, 0, 1)                                 # g0 b0
        ldu(nc.sync, 1, 2)                                   # g0 b1
        ldu(nc.scalar, 4, 6)                                 # g1 b0-1
        ldu(nc.sync, 2, 4)                                   # g0 b2-3
        nc.sync.dma_start(w_sb[:, kw:1824], wt[:, kw:1824])  # g1-5 weights
        nc.scalar.dma_start(b_sb[:], bt[:])
        ldu(nc.scalar, 6, 8)                                 # g1 b2-3

        # HAM warm-up: garbage matmuls with no DMA dependency keep the PE
        # busy from the preamble barrier until real data lands, so the
        # clock-gate releases (1.2 -> 2.4 GHz) as real matmuls begin
        for _ in range(NDUMMY):
            psd = pspool.tile([O, T], f32, tag="ps")
            nc.tensor.matmul(psd[:], scr[:, :O], scr[:, O:O + T],
                             start=True, stop=True)

        def main_group(i):
            # b-outer, k-inner: each batch's 3 taps run back-to-back, so
            # early batches start as soon as their own x unit lands (the
            # first 6 matmuls of group 0 need only units 0-1), and each
            # PSUM bank is ready for its bias-add after 3 matmuls
            o_sb = opool.tile([O, B * T], f16, tag="o")
            last = (i == GPC - 1)
            for b in range(B):
                ps = pspool.tile([O, T], f32, tag="ps", name=f"ps{b}")
                u = 4 * i + b
                for k in range(K):
                    nc.tensor.matmul(
                        ps[:],
                        w_sb[:, (i * K + k) * O:(i * K + k + 1) * O],
                        x_sb[:, u * TP + k:u * TP + k + T],
                        start=(k == 0),
                        stop=(k == K - 1),
                    )
                dst = o_sb[:, b * T:(b + 1) * T]
                if last and b == B - 1:
                    # final bias-add halved across both engines so the
                    # kernel-ending store chain starts ~0.4us sooner
                    nc.scalar.add(dst[:, :T // 2], ps[:, :T // 2],
                                  b_sb[:, i:i + 1])
                    nc.vector.tensor_scalar_add(dst[:, T // 2:],
                                                ps[:, T // 2:],
                                                b_sb[:, i:i + 1])
                elif b % 2 == 0:
                    nc.scalar.add(dst, ps[:], b_sb[:, i:i + 1])
                else:
                    nc.vector.tensor_scalar_add(dst, ps[:],
                                                b_sb[:, i:i + 1])
            c0 = i * B * T
            # stores are scalar's last op per group, gated only by this
            # group's own biases — nothing urgent ever queues behind; the
            # final group stores in halves on both rings (shorter drain)
            if last:
                h = B * T // 2
                nc.scalar.dma_start(om[:, c0:c0 + h], o_sb[:, :h])
                nc.sync.dma_start(om[:, c0 + h:c0 + B * T], o_sb[:, h:])
            else:
                nc.scalar.dma_start(om[:, c0:c0 + B * T], o_sb[:])

        def tail_group():
            ps = pspool.tile([O, TE], f32, tag="ps")
            xe0 = GPC * B * TP
            for k in range(K):
                nc.tensor.matmul(
                    ps[:],
                    w_sb[:, (GPC * K + k) * O:(GPC * K + k + 1) * O],
                    x_sb[:, xe0 + k:xe0 + k + TE],
                    start=(k == 0),
                    stop=(k == K - 1),
                )
            oe_sb = wpool.tile([O, TE], f16)
            nc.vector.tensor_scalar_add(oe_sb[:], ps[:],
                                        b_sb[:, GPC:GPC + 1])
            nc.sync.dma_start(om[:, GPC * B * T:], oe_sb[:])

        for i in range(GPC):
            # stream in x for group i+4 plus remaining weight pieces.
            # Loads alternate sync (HWDGE) / gpsimd (SWDGE): two queues
            # keep a 2/3 packet-round-robin share against the store
            # queue, and neither engine runs compute, so a load issue
            # blocked on semaphore-lane reuse can never stall a bias-add
            # behind it (that cascade stalls PSUM recycling and the PE)
            if i == 0:
                ldu(nc.sync, 8, 12)                              # g2
                ldu(nc.gpsimd, 12, 16)                           # g3
                nc.sync.dma_start(w_sb[:, 1824:3360],            # g6-10 w
                                  wt[:, 1824:3360])
            elif i == 1:
                nc.sync.dma_start(w_sb[:, 3360:], wt[:, 3360:])  # g11-16+tail
            elif i == 2:
                nc.sync.dma_start(x_sb[:, GPC * B * TP:],        # tail-group x
                                  xm[:, GPC * B * TP:])
            if i <= GPC - 5:
                ldu(nc.sync if i % 2 == 0 else nc.gpsimd,
                    4 * (i + 4), 4 * (i + 5))
            if i == GPC - 1:
                # tiny shared-group chunk right before the final main
                # group: its small store drains alongside the final ones
                tail_group()
            main_group(i)

    nc.finalize()
    return nc


def shard_inputs(x, weight, bias):
    x = np.ascontiguousarray(x, dtype=np.float32)
    weight = np.ascontiguousarray(weight, dtype=np.float32)
    bias = np.ascontiguousarray(bias, dtype=np.float32)

    xp = np.pad(x, ((0, 0), (1, 1), (0, 0), (0, 0)))          # [B, TP, G, F]
    xt = xp.transpose(2, 3, 0, 1).astype(np.float16)          # [G, F, B, TP]
    # weight [G, O, F, K] -> [F, G, K, O]
    wtr = weight.transpose(2, 0, 3, 1).astype(np.float16)

    in_maps = []
    for c in range(NCORES):
        gs = list(range(c * GPC, (c + 1) * GPC)) + [G - 1]
        b_c, t0 = c // 2, (c % 2) * TE
        xm_c = np.empty((F, XC), np.float16)
        xm_c[:, :GPC * B * TP] = (
            xt[c * GPC:(c + 1) * GPC].transpose(1, 0, 2, 3)
            .reshape(F, GPC * B * TP))
        xm_c[:, GPC * B * TP:] = xt[G - 1, :, b_c, t0:t0 + TEP]
        in_maps.append({
            "xm": xm_c,
            "wt": np.ascontiguousarray(wtr[:, gs].reshape(F, WC)),
            "bt": np.ascontiguousarray(bias[gs].T),
            })
    return in_maps


def unshard_outputs(results):
    out = np.empty((B, T, G, O), dtype=np.float32)
    for c in range(NCORES):
        om = results[c]["om"].astype(np.float32)              # [O, OC]
        main = om[:, :GPC * B * T].reshape(O, GPC, B, T)
        out[:, :, c * GPC:(c + 1) * GPC, :] = main.transpose(2, 3, 1, 0)
        b_c, t0 = c // 2, (c % 2) * TE
        out[b_c, t0:t0 + TE, G - 1, :] = om[:, GPC * B * T:].T
    return out


def run(x, weight, bias, **run_kwargs):
    nc = build_program()
    in_maps = shard_inputs(x, weight, bias)
    res = run_bass_kernel_spmd(nc, in_maps, list(range(NCORES)), **run_kwargs)
    return unshard_outputs(res.results), res


def kernel(x, weight, bias):
    out, _ = run(x, weight, bias)
    return out


# revision 23
# speedup vs baseline: 1.1209x; 1.1209x over previous
"""Grouped Conv1d (B=4, T=512, G=129, F=96 -> O=96, K=3, pad=1) on 8 trn2 cores.

Sharding: 129 groups = 16 full groups per core + group 128 split across all
8 cores by (batch b = core//2, T-half = core%2).  SPMD: every core runs the
identical program on its own slice.

Per (group, batch): out[o, t] = sum_k w_k[f, o].T @ x[f, t+k-1]  (3 matmuls
accumulated in fp32 PSUM).  x and w are cast to fp16 on the host: fp16 runs
the PE moving operand at full rate, halves the x DMA bytes, and keeps max
rel err ~5e-4 (accumulate stays fp32).  Bias is added fp32 on ScalarE /
VectorE (alternating) while copying PSUM -> SBUF; output stored fp16.

Layout: one flat x tile [96, 33154] covering all 16 padded (g,b) units plus
the tail-group slice lives in SBUF for the whole kernel (66 KB/partition).
DMA rides the two HWDGE rings with long per-partition lines (first pieces
small for latency, then 16 KB lines = 4x 4096B packets + 64B runt, ~2%
packet overhead instead of the 20% a 4112B line pays).  Output stores are
exactly 4096B/partition per group.  A short burst of dummy matmuls on a
memset scratch tile runs during the first-DMA latency window so the PE HAM
clock-gate is warm (2.4 GHz) when real matmuls start.
"""

from contextlib import ExitStack

import numpy as np

import concourse.bass as bass
import concourse.mybir as mybir
import concourse.tile as tile
from concourse import bacc
from concourse.bass_utils import run_bass_kernel_spmd

B, T, G, F, O, K = 4, 512, 129, 96, 96, 3
NCORES = 8
GPC = 16          # full groups per core (8*16 = 128; group 128 is split 8 ways)
TP = T + 2        # padded unit length per (g, b)
TE = T // 2       # tail-group T chunk per core
TEP = TE + 2
XC = GPC * B * TP + TEP      # x columns per core (33154)
OC = GPC * B * T + TE        # out columns per core (33024)
WC = (GPC + 1) * K * O       # weight columns per core (4896)
NDUMMY = 6                   # HAM warm-up matmuls


def build_program():
    nc = bacc.Bacc("TRN2", target_bir_lowering=False, debug=False,
                   num_devices=NCORES)

    f32 = mybir.dt.float32
    f16 = mybir.dt.float16

    xm = nc.dram_tensor("xm", [F, XC], f16, kind="ExternalInput")
    wt = nc.dram_tensor("wt", [F, WC], f16, kind="ExternalInput")
    bt = nc.dram_tensor("bt", [O, GPC + 1], f32, kind="ExternalInput")
    om = nc.dram_tensor("om", [O, OC], f16, kind="ExternalOutput")

    with ExitStack() as ctx:
        tc = ctx.enter_context(tile.TileContext(nc))
        wpool = ctx.enter_context(tc.tile_pool(name="w", bufs=1))
        opool = ctx.enter_context(tc.tile_pool(name="o", bufs=4))
        pspool = ctx.enter_context(tc.tile_pool(name="ps", bufs=8, space="PSUM"))

        x_sb = wpool.tile([F, XC], f16)
        w_sb = wpool.tile([F, WC], f16)
        b_sb = wpool.tile([O, GPC + 1], f32)
        scr = wpool.tile([F, O + T], f16)      # dummy-matmul operands

        nc.vector.memset(scr[:], 0.0)

        def ldu(eng, u0, u1):                  # load x units [u0, u1)
            eng.dma_start(x_sb[:, u0 * TP:u1 * TP], xm[:, u0 * TP:u1 * TP])

        kw = K * O
        # prologue: at most 8 outstanding HWDGE DMAs (8 completion-sem
        # lanes exist; a 9th dma_start would stall its whole engine queue
        # on lane reuse).  Tiny interleaved first pieces so the early
        # per-ring completion pipeline (one sem per ~1us) matches the
        # cold-matmul consumption order; later stripes are issued from
        # inside the compute loop on the sync ring only — sync runs no
        # compute, so stripe issues never queue behind a bias-add wait
        nc.sync.dma_start(w_sb[:, :kw], wt[:, :kw])          # g0 weights
        ldu(nc.scalar, 0, 1)                                 # g0 b0
        ldu(nc.sync, 1, 2)                                   # g0 b1
        ldu(nc.scalar, 4, 6)                                 # g1 b0-1
        ldu(nc.sync, 2, 4)                                   # g0 b2-3
        nc.sync.dma_start(w_sb[:, kw:1824], wt[:, kw:1824])  # g1-5 weights
        nc.scalar.dma_start(b_sb[:], bt[:])
        ldu(nc.scalar, 6, 8)                                 # g1 b2-3

        # HAM warm-up: garbage matmuls with no DMA dependency keep the PE
        # busy from the preamble barrier until real data lands, so the
        # clock-gate releases (1.2 -> 2.4 GHz) as real matmuls begin
        for _ in range(NDUMMY):
            psd = pspool.tile([O, T], f32, tag="ps")
            nc.tensor.matmul(psd[:], scr[:, :O], scr[:, O:O + T],
                             start=True, stop=True)

        def main_group(i):
            # b-outer, k-inner: each batch's 3 taps run back-to-back, so
            # early batches start as soon as their own x unit lands (the
            # first 6 matmuls of group 0 need only units 0-1), and each
            # PSUM bank is ready for its bias-add after 3 matmuls
            o_sb = opool.tile([O, B * T], f16, tag="o")
            last = (i == GPC - 1)
            for b in range(B):
                ps = pspool.tile([O, T], f32, tag="ps", name=f"ps{b}")
                u = 4 * i + b
                for k in range(K):
                    nc.tensor.matmul(
                        ps[:],
                        w_sb[:, (i * K + k) * O:(i * K + k + 1) * O],
                        x_sb[:, u * TP + k:u * TP + k + T],
                        start=(k == 0),
                        stop=(k == K - 1),
                    )
                dst = o_sb[:, b * T:(b + 1) * T]
                if last and b == B - 1:
                    # final bias-add halved across both engines so the
                    # kernel-ending store chain starts ~0.4us sooner
                    nc.scalar.add(dst[:, :T // 2], ps[:, :T // 2],
                                  b_sb[:, i:i + 1])
                    nc.vector.tensor_scalar_add(dst[:, T // 2:],
                                                ps[:, T // 2:],
                                                b_sb[:, i:i + 1])
                elif b % 2 == 0:
                    nc.scalar.add(dst, ps[:], b_sb[:, i:i + 1])
                else:
                    nc.vector.tensor_scalar_add(dst, ps[:],
                                                b_sb[:, i:i + 1])
            c0 = i * B * T
            # stores are scalar's last op per group, gated only by this
            # group's own biases — nothing urgent ever queues behind; the
            # final group stores in halves on both rings (shorter drain)
            if last:
                h = B * T // 2
                nc.scalar.dma_start(om[:, c0:c0 + h], o_sb[:, :h])
                nc.sync.dma_start(om[:, c0 + h:c0 + B * T], o_sb[:, h:])
            else:
                nc.scalar.dma_start(om[:, c0:c0 + B * T], o_sb[:])

        def tail_group():
            ps = pspool.tile([O, TE], f32, tag="ps")
            xe0 = GPC * B * TP
            for k in range(K):
                nc.tensor.matmul(
                    ps[:],
                    w_sb[:, (GPC * K + k) * O:(GPC * K + k + 1) * O],
                    x_sb[:, xe0 + k:xe0 + k + TE],
                    start=(k == 0),
                    stop=(k == K - 1),
                )
            oe_sb = wpool.tile([O, TE], f16)
            nc.vector.tensor_scalar_add(oe_sb[:], ps[:],
                                        b_sb[:, GPC:GPC + 1])
            nc.scalar.dma_start(om[:, GPC * B * T:], oe_sb[:])

        for i in range(GPC):
            # stream in x aggressively on sync — a pure load queue whose
            # only waits are sem-lane reuse (self-pacing at 8 in flight)
            # and which therefore can never stall compute; front-loading
            # builds a multi-group cushion while the store queue is idle
            if i == 0:
                ldu(nc.sync, 8, 12)                              # g2
                nc.sync.dma_start(w_sb[:, 1824:3360],            # g6-10 w
                                  wt[:, 1824:3360])
                ldu(nc.sync, 12, 16)                             # g3
            elif i == 1:
                nc.sync.dma_start(w_sb[:, 3360:], wt[:, 3360:])  # g11-16+tail
            elif i == 2:
                nc.sync.dma_start(x_sb[:, GPC * B * TP:],        # tail-group x
                                  xm[:, GPC * B * TP:])
            for uu in (2 * i + 4, 2 * i + 5):
                if 4 <= uu < GPC:
                    ldu(nc.sync, 4 * uu, 4 * (uu + 1))
            if i == GPC - 1:
                # tiny shared-group chunk right before the final main
                # group: its small store drains alongside the final ones
                tail_group()
            main_group(i)

    nc.finalize()
    return nc


def shard_inputs(x, weight, bias):
    x = np.ascontiguousarray(x, dtype=np.float32)
    weight = np.ascontiguousarray(weight, dtype=np.float32)
    bias = np.ascontiguousarray(bias, dtype=np.float32)

    xp = np.pad(x, ((0, 0), (1, 1), (0, 0), (0, 0)))          # [B, TP, G, F]
    xt = xp.transpose(2, 3, 0, 1).astype(np.float16)          # [G, F, B, TP]
    # weight [G, O, F, K] -> [F, G, K, O]
    wtr = weight.transpose(2, 0, 3, 1).astype(np.float16)

    in_maps = []
    for c in range(NCORES):
        gs = list(range(c * GPC, (c + 1) * GPC)) + [G - 1]
        b_c, t0 = c // 2, (c % 2) * TE
        xm_c = np.empty((F, XC), np.float16)
        xm_c[:, :GPC * B * TP] = (
            xt[c * GPC:(c + 1) * GPC].transpose(1, 0, 2, 3)
            .reshape(F, GPC * B * TP))
        xm_c[:, GPC * B * TP:] = xt[G - 1, :, b_c, t0:t0 + TEP]
        in_maps.append({
            "xm": xm_c,
            "wt": np.ascontiguousarray(wtr[:, gs].reshape(F, WC)),
            "bt": np.ascontiguousarray(bias[gs].T),
            })
    return in_maps


def unshard_outputs(results):
    out = np.empty((B, T, G, O), dtype=np.float32)
    for c in range(NCORES):
        om = results[c]["om"].astype(np.float32)              # [O, OC]
        main = om[:, :GPC * B * T].reshape(O, GPC, B, T)
        out[:, :, c * GPC:(c + 1) * GPC, :] = main.transpose(2, 3, 1, 0)
        b_c, t0 = c // 2, (c % 2) * TE
        out[b_c, t0:t0 + TE, G - 1, :] = om[:, GPC * B * T:].T
    return out


def run(x, weight, bias, **run_kwargs):
    nc = build_program()
    in_maps = shard_inputs(x, weight, bias)
    res = run_bass_kernel_spmd(nc, in_maps, list(range(NCORES)), **run_kwargs)
    return unshard_outputs(res.results), res


def kernel(x, weight, bias):
    out, _ = run(x, weight, bias)
    return out


# revision 26
# speedup vs baseline: 1.1504x; 1.0263x over previous
"""Grouped Conv1d (B=4, T=512, G=129, F=96 -> O=96, K=3, pad=1) on 8 trn2 cores.

Sharding: 129 groups = 16 full groups per core + group 128 split across all
8 cores by (batch b = core//2, T-half = core%2).  SPMD: every core runs the
identical program on its own slice.

Per (group, batch): out[o, t] = sum_k w_k[f, o].T @ x[f, t+k-1]  (3 matmuls
accumulated in fp32 PSUM).  x and w are cast to fp16 on the host: fp16 runs
the PE moving operand at full rate, halves the x DMA bytes, and keeps max
rel err ~5e-4 (accumulate stays fp32).  Bias is added fp32 on ScalarE /
VectorE (alternating) while copying PSUM -> SBUF; output stored fp16.

Layout: one flat x tile [96, 33154] covering all 16 padded (g,b) units plus
the tail-group slice lives in SBUF for the whole kernel (66 KB/partition).
DMA rides the two HWDGE rings with long per-partition lines (first pieces
small for latency, then 16 KB lines = 4x 4096B packets + 64B runt, ~2%
packet overhead instead of the 20% a 4112B line pays).  Output stores are
exactly 4096B/partition per group.  A short burst of dummy matmuls on a
memset scratch tile runs during the first-DMA latency window so the PE HAM
clock-gate is warm (2.4 GHz) when real matmuls start.
"""

from contextlib import ExitStack

import numpy as np

import concourse.bass as bass
import concourse.mybir as mybir
import concourse.tile as tile
from concourse import bacc
from concourse.bass_utils import run_bass_kernel_spmd

B, T, G, F, O, K = 4, 512, 129, 96, 96, 3
NCORES = 8
GPC = 16          # full groups per core (8*16 = 128; group 128 is split 8 ways)
TP = T + 2        # padded unit length per (g, b)
TE = T // 2       # tail-group T chunk per core
TEP = TE + 2
XC = GPC * B * TP + TEP      # x columns per core (33154)
OC = GPC * B * T + TE        # out columns per core (33024)
WC = (GPC + 1) * K * O       # weight columns per core (4896)
NDUMMY = 6                   # HAM warm-up matmuls


def build_program():
    nc = bacc.Bacc("TRN2", target_bir_lowering=False, debug=False,
                   num_devices=NCORES)

    f32 = mybir.dt.float32
    f16 = mybir.dt.float16

    xm = nc.dram_tensor("xm", [F, XC], f16, kind="ExternalInput")
    wt = nc.dram_tensor("wt", [F, WC], f16, kind="ExternalInput")
    bt = nc.dram_tensor("bt", [O, GPC + 1], f32, kind="ExternalInput")
    om = nc.dram_tensor("om", [O, OC], f16, kind="ExternalOutput")

    with ExitStack() as ctx:
        tc = ctx.enter_context(tile.TileContext(nc))
        wpool = ctx.enter_context(tc.tile_pool(name="w", bufs=1))
        opool = ctx.enter_context(tc.tile_pool(name="o", bufs=4))
        pspool = ctx.enter_context(tc.tile_pool(name="ps", bufs=8, space="PSUM"))

        x_sb = wpool.tile([F, XC], f16)
        w_sb = wpool.tile([F, WC], f16)
        b_sb = wpool.tile([O, GPC + 1], f32)
        scr = wpool.tile([F, O + T], f16)      # dummy-matmul operands

        nc.vector.memset(scr[:], 0.0)

        def ldu(eng, u0, u1):                  # load x units [u0, u1)
            eng.dma_start(x_sb[:, u0 * TP:u1 * TP], xm[:, u0 * TP:u1 * TP])

        kw = K * O
        # prologue: at most 8 outstanding HWDGE DMAs (8 completion-sem
        # lanes exist; a 9th dma_start would stall its whole engine queue
        # on lane reuse).  Tiny interleaved first pieces so the early
        # per-ring completion pipeline (one sem per ~1us) matches the
        # cold-matmul consumption order; later stripes are issued from
        # inside the compute loop on the sync ring only — sync runs no
        # compute, so stripe issues never queue behind a bias-add wait
        nc.sync.dma_start(w_sb[:, :kw], wt[:, :kw])          # g0 weights
        ldu(nc.scalar, 0, 1)                                 # g0 b0
        ldu(nc.sync, 1, 2)                                   # g0 b1
        ldu(nc.scalar, 4, 6)                                 # g1 b0-1
        ldu(nc.sync, 2, 4)                                   # g0 b2-3
        nc.sync.dma_start(w_sb[:, kw:2 * kw], wt[:, kw:2 * kw])  # g1 weights
        nc.scalar.dma_start(b_sb[:], bt[:])
        ldu(nc.scalar, 6, 8)                                 # g1 b2-3

        # HAM warm-up: garbage matmuls with no DMA dependency keep the PE
        # busy from the preamble barrier until real data lands, so the
        # clock-gate releases (1.2 -> 2.4 GHz) as real matmuls begin
        for _ in range(NDUMMY):
            psd = pspool.tile([O, T], f32, tag="ps")
            nc.tensor.matmul(psd[:], scr[:, :O], scr[:, O:O + T],
                             start=True, stop=True)

        def main_group(i):
            # b-outer, k-inner: each batch's 3 taps run back-to-back, so
            # early batches start as soon as their own x unit lands (the
            # first 6 matmuls of group 0 need only units 0-1), and each
            # PSUM bank is ready for its bias-add after 3 matmuls
            o_sb = opool.tile([O, B * T], f16, tag="o")
            last = (i == GPC - 1)
            for b in range(B):
                ps = pspool.tile([O, T], f32, tag="ps", name=f"ps{b}")
                u = 4 * i + b
                for k in range(K):
                    nc.tensor.matmul(
                        ps[:],
                        w_sb[:, (i * K + k) * O:(i * K + k + 1) * O],
                        x_sb[:, u * TP + k:u * TP + k + T],
                        start=(k == 0),
                        stop=(k == K - 1),
                    )
                dst = o_sb[:, b * T:(b + 1) * T]
                if last and b == B - 1:
                    # final bias-add halved across both engines so the
                    # kernel-ending store chain starts ~0.4us sooner
                    nc.scalar.add(dst[:, :T // 2], ps[:, :T // 2],
                                  b_sb[:, i:i + 1])
                    nc.vector.tensor_scalar_add(dst[:, T // 2:],
                                                ps[:, T // 2:],
                                                b_sb[:, i:i + 1])
                elif last and b == B - 2:
                    # vector takes b2 too: scalar is busy with the final
                    # stores, vector would otherwise idle here
                    nc.vector.tensor_scalar_add(dst, ps[:],
                                                b_sb[:, i:i + 1])
                elif b % 2 == 0:
                    nc.scalar.add(dst, ps[:], b_sb[:, i:i + 1])
                else:
                    nc.vector.tensor_scalar_add(dst, ps[:],
                                                b_sb[:, i:i + 1])
            c0 = i * B * T
            # stores are scalar's last op per group, gated only by this
            # group's own biases — nothing urgent ever queues behind; the
            # final group stores in halves on both rings (shorter drain)
            if last:
                h = B * T // 2
                nc.scalar.dma_start(om[:, c0:c0 + h], o_sb[:, :h])
                nc.sync.dma_start(om[:, c0 + h:c0 + B * T], o_sb[:, h:])
            else:
                nc.scalar.dma_start(om[:, c0:c0 + B * T], o_sb[:])

        def tail_group():
            ps = pspool.tile([O, TE], f32, tag="ps")
            xe0 = GPC * B * TP
            for k in range(K):
                nc.tensor.matmul(
                    ps[:],
                    w_sb[:, (GPC * K + k) * O:(GPC * K + k + 1) * O],
                    x_sb[:, xe0 + k:xe0 + k + TE],
                    start=(k == 0),
                    stop=(k == K - 1),
                )
            oe_sb = wpool.tile([O, TE], f16)
            nc.vector.tensor_scalar_add(oe_sb[:], ps[:],
                                        b_sb[:, GPC:GPC + 1])
            nc.scalar.dma_start(om[:, GPC * B * T:], oe_sb[:])

        for i in range(GPC):
            # stream in x aggressively on sync — a pure load queue whose
            # only waits are sem-lane reuse (self-pacing at 8 in flight)
            # and which therefore can never stall compute; front-loading
            # builds a multi-group cushion while the store queue is idle
            if i == 0:
                nc.sync.dma_start(w_sb[:, 2 * kw:1824],          # g2-5 w
                                  wt[:, 2 * kw:1824])
                ldu(nc.sync, 8, 12)                              # g2
                nc.sync.dma_start(w_sb[:, 1824:3360],            # g6-10 w
                                  wt[:, 1824:3360])
                ldu(nc.sync, 12, 16)                             # g3
            elif i == 1:
                nc.sync.dma_start(w_sb[:, 3360:], wt[:, 3360:])  # g11-16+tail
            elif i == 2:
                nc.sync.dma_start(x_sb[:, GPC * B * TP:],        # tail-group x
                                  xm[:, GPC * B * TP:])
            for uu in (2 * i + 4, 2 * i + 5):
                if 4 <= uu < GPC:
                    ldu(nc.sync, 4 * uu, 4 * (uu + 1))
            if i == GPC - 1:
                # tiny shared-group chunk right before the final main
                # group: its small store drains alongside the final ones
                tail_group()
            main_group(i)

    nc.finalize()
    return nc


def shard_inputs(x, weight, bias):
    x = np.ascontiguousarray(x, dtype=np.float32)
    weight = np.ascontiguousarray(weight, dtype=np.float32)
    bias = np.ascontiguousarray(bias, dtype=np.float32)

    xp = np.pad(x, ((0, 0), (1, 1), (0, 0), (0, 0)))          # [B, TP, G, F]
    xt = xp.transpose(2, 3, 0, 1).astype(np.float16)          # [G, F, B, TP]
    # weight [G, O, F, K] -> [F, G, K, O]
    wtr = weight.transpose(2, 0, 3, 1).astype(np.float16)

    in_maps = []
    for c in range(NCORES):
        gs = list(range(c * GPC, (c + 1) * GPC)) + [G - 1]
        b_c, t0 = c // 2, (c % 2) * TE
        xm_c = np.empty((F, XC), np.float16)
        xm_c[:, :GPC * B * TP] = (
            xt[c * GPC:(c + 1) * GPC].transpose(1, 0, 2, 3)
            .reshape(F, GPC * B * TP))
        xm_c[:, GPC * B * TP:] = xt[G - 1, :, b_c, t0:t0 + TEP]
        in_maps.append({
            "xm": xm_c,
            "wt": np.ascontiguousarray(wtr[:, gs].reshape(F, WC)),
            "bt": np.ascontiguousarray(bias[gs].T),
            })
    return in_maps


def unshard_outputs(results):
    out = np.empty((B, T, G, O), dtype=np.float32)
    for c in range(NCORES):
        om = results[c]["om"].astype(np.float32)              # [O, OC]
        main = om[:, :GPC * B * T].reshape(O, GPC, B, T)
        out[:, :, c * GPC:(c + 1) * GPC, :] = main.transpose(2, 3, 1, 0)
        b_c, t0 = c // 2, (c % 2) * TE
        out[b_c, t0:t0 + TE, G - 1, :] = om[:, GPC * B * T:].T
    return out


def run(x, weight, bias, **run_kwargs):
    nc = build_program()
    in_maps = shard_inputs(x, weight, bias)
    res = run_bass_kernel_spmd(nc, in_maps, list(range(NCORES)), **run_kwargs)
    return unshard_outputs(res.results), res


def kernel(x, weight, bias):
    out, _ = run(x, weight, bias)
    return out


# revision 29
# speedup vs baseline: 1.1543x; 1.0034x over previous
"""Grouped Conv1d (B=4, T=512, G=129, F=96 -> O=96, K=3, pad=1) on 8 trn2 cores.

Sharding: 129 groups = 16 full groups per core + group 128 split across all
8 cores by (batch b = core//2, T-half = core%2).  SPMD: every core runs the
identical program on its own slice.

Per (group, batch): out[o, t] = sum_k w_k[f, o].T @ x[f, t+k-1]  (3 matmuls
accumulated in fp32 PSUM).  x and w are cast to fp16 on the host: fp16 runs
the PE moving operand at full rate, halves the x DMA bytes, and keeps max
rel err ~5e-4 (accumulate stays fp32).  Bias is added fp32 on ScalarE /
VectorE (alternating) while copying PSUM -> SBUF; output stored fp16.

Layout: one flat x tile [96, 33154] covering all 16 padded (g,b) units plus
the tail-group slice lives in SBUF for the whole kernel (66 KB/partition).
DMA rides the two HWDGE rings with long per-partition lines (first pieces
small for latency, then 16 KB lines = 4x 4096B packets + 64B runt, ~2%
packet overhead instead of the 20% a 4112B line pays).  Output stores are
exactly 4096B/partition per group.  A short burst of dummy matmuls on a
memset scratch tile runs during the first-DMA latency window so the PE HAM
clock-gate is warm (2.4 GHz) when real matmuls start.
"""

from contextlib import ExitStack

import numpy as np

import concourse.bass as bass
import concourse.mybir as mybir
import concourse.tile as tile
from concourse import bacc
from concourse.bass_utils import run_bass_kernel_spmd

B, T, G, F, O, K = 4, 512, 129, 96, 96, 3
NCORES = 8
GPC = 16          # full groups per core (8*16 = 128; group 128 is split 8 ways)
TP = T + 2        # padded unit length per (g, b)
TE = T // 2       # tail-group T chunk per core
TEP = TE + 2
XC = GPC * B * TP + TEP      # x columns per core (33154)
OC = GPC * B * T + TE        # out columns per core (33024)
WC = (GPC + 1) * K * O       # weight columns per core (4896)
NDUMMY = 6                   # HAM warm-up matmuls


def build_program():
    nc = bacc.Bacc("TRN2", target_bir_lowering=False, debug=False,
                   num_devices=NCORES)

    f32 = mybir.dt.float32
    f16 = mybir.dt.float16

    xm = nc.dram_tensor("xm", [F, XC], f16, kind="ExternalInput")
    wt = nc.dram_tensor("wt", [F, WC], f16, kind="ExternalInput")
    bt = nc.dram_tensor("bt", [O, GPC + 1], f32, kind="ExternalInput")
    om = nc.dram_tensor("om", [O, OC], f16, kind="ExternalOutput")

    with ExitStack() as ctx:
        tc = ctx.enter_context(tile.TileContext(nc))
        wpool = ctx.enter_context(tc.tile_pool(name="w", bufs=1))
        opool = ctx.enter_context(tc.tile_pool(name="o", bufs=4))
        pspool = ctx.enter_context(tc.tile_pool(name="ps", bufs=8, space="PSUM"))

        x_sb = wpool.tile([F, XC], f16)
        w_sb = wpool.tile([F, WC], f16)
        b_sb = wpool.tile([O, GPC + 1], f32)
        scr = wpool.tile([F, O + T], f16)      # dummy-matmul operands

        nc.vector.memset(scr[:], 0.0)

        def ldu(eng, u0, u1):                  # load x units [u0, u1)
            eng.dma_start(x_sb[:, u0 * TP:u1 * TP], xm[:, u0 * TP:u1 * TP])

        kw = K * O
        # prologue: at most 8 outstanding HWDGE DMAs (8 completion-sem
        # lanes exist; a 9th dma_start would stall its whole engine queue
        # on lane reuse).  Tiny interleaved first pieces so the early
        # per-ring completion pipeline (one sem per ~1us) matches the
        # cold-matmul consumption order; later stripes are issued from
        # inside the compute loop on the sync ring only — sync runs no
        # compute, so stripe issues never queue behind a bias-add wait
        nc.sync.dma_start(w_sb[:, :kw], wt[:, :kw])          # g0 weights
        ldu(nc.scalar, 0, 1)                                 # g0 b0
        ldu(nc.sync, 1, 2)                                   # g0 b1
        ldu(nc.scalar, 4, 6)                                 # g1 b0-1
        ldu(nc.sync, 2, 4)                                   # g0 b2-3
        nc.sync.dma_start(w_sb[:, kw:2 * kw], wt[:, kw:2 * kw])  # g1 weights
        nc.scalar.dma_start(b_sb[:], bt[:])
        ldu(nc.scalar, 6, 8)                                 # g1 b2-3

        # HAM warm-up: garbage matmuls with no DMA dependency keep the PE
        # busy from the preamble barrier until real data lands, so the
        # clock-gate releases (1.2 -> 2.4 GHz) as real matmuls begin
        for _ in range(NDUMMY):
            psd = pspool.tile([O, T], f32, tag="ps")
            nc.tensor.matmul(psd[:], scr[:, :O], scr[:, O:O + T],
                             start=True, stop=True)

        def main_group(i):
            # b-outer, k-inner: each batch's 3 taps run back-to-back, so
            # early batches start as soon as their own x unit lands (the
            # first 6 matmuls of group 0 need only units 0-1), and each
            # PSUM bank is ready for its bias-add after 3 matmuls
            o_sb = opool.tile([O, B * T], f16, tag="o")
            last = (i == GPC - 1)
            for b in range(B):
                ps = pspool.tile([O, T], f32, tag="ps", name=f"ps{b}")
                u = 4 * i + b
                for k in range(K):
                    nc.tensor.matmul(
                        ps[:],
                        w_sb[:, (i * K + k) * O:(i * K + k + 1) * O],
                        x_sb[:, u * TP + k:u * TP + k + T],
                        start=(k == 0),
                        stop=(k == K - 1),
                    )
                dst = o_sb[:, b * T:(b + 1) * T]
                if last and b >= B - 2:
                    # final two bias-adds halved across both engines so
                    # the kernel-ending store chain starts ~1us sooner
                    nc.scalar.add(dst[:, :T // 2], ps[:, :T // 2],
                                  b_sb[:, i:i + 1])
                    nc.vector.tensor_scalar_add(dst[:, T // 2:],
                                                ps[:, T // 2:],
                                                b_sb[:, i:i + 1])
                elif b % 2 == 0:
                    nc.scalar.add(dst, ps[:], b_sb[:, i:i + 1])
                else:
                    nc.vector.tensor_scalar_add(dst, ps[:],
                                                b_sb[:, i:i + 1])
            c0 = i * B * T
            # stores are scalar's last op per group, gated only by this
            # group's own biases — nothing urgent ever queues behind; the
            # final group stores in halves on both rings (shorter drain)
            if last:
                h = B * T // 2
                nc.scalar.dma_start(om[:, c0:c0 + h], o_sb[:, :h])
                nc.sync.dma_start(om[:, c0 + h:c0 + B * T], o_sb[:, h:])
            else:
                nc.scalar.dma_start(om[:, c0:c0 + B * T], o_sb[:])

        def tail_group():
            ps = pspool.tile([O, TE], f32, tag="ps")
            xe0 = GPC * B * TP
            for k in range(K):
                nc.tensor.matmul(
                    ps[:],
                    w_sb[:, (GPC * K + k) * O:(GPC * K + k + 1) * O],
                    x_sb[:, xe0 + k:xe0 + k + TE],
                    start=(k == 0),
                    stop=(k == K - 1),
                )
            oe_sb = wpool.tile([O, TE], f16)
            nc.vector.tensor_scalar_add(oe_sb[:], ps[:],
                                        b_sb[:, GPC:GPC + 1])
            nc.sync.dma_start(om[:, GPC * B * T:], oe_sb[:])

        for i in range(GPC):
            # stream in x aggressively on sync — a pure load queue whose
            # only waits are sem-lane reuse (self-pacing at 8 in flight)
            # and which therefore can never stall compute; front-loading
            # builds a multi-group cushion while the store queue is idle
            if i == 0:
                nc.sync.dma_start(w_sb[:, 2 * kw:1824],          # g2-5 w
                                  wt[:, 2 * kw:1824])
                ldu(nc.sync, 8, 12)                              # g2
                nc.sync.dma_start(w_sb[:, 1824:3360],            # g6-10 w
                                  wt[:, 1824:3360])
                ldu(nc.sync, 12, 16)                             # g3
            elif i == 1:
                nc.sync.dma_start(w_sb[:, 3360:], wt[:, 3360:])  # g11-16+tail
            elif i == 2:
                nc.sync.dma_start(x_sb[:, GPC * B * TP:],        # tail-group x
                                  xm[:, GPC * B * TP:])
            for uu in (2 * i + 4, 2 * i + 5):
                if 4 <= uu < GPC:
                    ldu(nc.sync, 4 * uu, 4 * (uu + 1))
            main_group(i)
        # tiny shared-group chunk closes the kernel: its 3 short matmuls
        # and small store issue while the final big store's receipt is
        # already in flight, so both drains overlap
        tail_group()

    nc.finalize()
    return nc


def shard_inputs(x, weight, bias):
    x = np.ascontiguousarray(x, dtype=np.float32)
    weight = np.ascontiguousarray(weight, dtype=np.float32)
    bias = np.ascontiguousarray(bias, dtype=np.float32)

    xp = np.pad(x, ((0, 0), (1, 1), (0, 0), (0, 0)))          # [B, TP, G, F]
    xt = xp.transpose(2, 3, 0, 1).astype(np.float16)          # [G, F, B, TP]
    # weight [G, O, F, K] -> [F, G, K, O]
    wtr = weight.transpose(2, 0, 3, 1).astype(np.float16)

    in_maps = []
    for c in range(NCORES):
        gs = list(range(c * GPC, (c + 1) * GPC)) + [G - 1]
        b_c, t0 = c // 2, (c % 2) * TE
        xm_c = np.empty((F, XC), np.float16)
        xm_c[:, :GPC * B * TP] = (
            xt[c * GPC:(c + 1) * GPC].transpose(1, 0, 2, 3)
            .reshape(F, GPC * B * TP))
        xm_c[:, GPC * B * TP:] = xt[G - 1, :, b_c, t0:t0 + TEP]
        in_maps.append({
            "xm": xm_c,
            "wt": np.ascontiguousarray(wtr[:, gs].reshape(F, WC)),
            "bt": np.ascontiguousarray(bias[gs].T),
            })
    return in_maps


def unshard_outputs(results):
    out = np.empty((B, T, G, O), dtype=np.float32)
    for c in range(NCORES):
        om = results[c]["om"].astype(np.float32)              # [O, OC]
        main = om[:, :GPC * B * T].reshape(O, GPC, B, T)
        out[:, :, c * GPC:(c + 1) * GPC, :] = main.transpose(2, 3, 1, 0)
        b_c, t0 = c // 2, (c % 2) * TE
        out[b_c, t0:t0 + TE, G - 1, :] = om[:, GPC * B * T:].T
    return out


def run(x, weight, bias, **run_kwargs):
    nc = build_program()
    in_maps = shard_inputs(x, weight, bias)
    res = run_bass_kernel_spmd(nc, in_maps, list(range(NCORES)), **run_kwargs)
    return unshard_outputs(res.results), res


def kernel(x, weight, bias):
    out, _ = run(x, weight, bias)
    return out
